# revision 22
# baseline (speedup 1.0000x reference)
"""AFGNN layer (6-hop sparse message passing + softmax mix + dense proj) on
8 TRN2 NeuronCores — v4.

vs v3 (1.77ms measured):
  - Tile-major edge order (64-row tiles, supertiles of 4): each tile's
    4 chunk contributions chain in ONE PSUM accumulator -> no repT SBUF
    accumulator, no f32 flush TTs, and epilogues stream throughout the
    kernel instead of serializing at the end.
  - P one-hot build is ONE fused custom-DVE instruction per gather call
    (eq(slot, Idx - PageIdx) * sval over [128, nblocks, 64]) instead of
    one STT per 128-edge block: DVE drops from ~1.6ms to ~0.3ms.
  - 64-wide row tiles halve the matmul streaming cols and the P
    elements vs 128.
  - Gather calls merged per (supertile, chunk) (~2-3k descriptors each)
    to amortize the 994ns SWDGE fixed overhead; greedy queue balance.
Gather stays SWDGE dma_gather; 256B descriptors cost ~26ns/engine
(sub-512B RMW penalty), so the DMA floor is ~0.7ms aggregate - the
expected new wall.
"""

import numpy as np
import ml_dtypes

N = 100000
NPAD = 100352          # 784 * 128
D = 128
NCORES = 8
RPC = NPAD // NCORES   # 12544 rows per core
TILEW = 128
NT = RPC // TILEW      # 98 row tiles per core
S = 14                 # tiles per supertile
NST = NT // S          # 7
NCH = 4
CH_BASE = (0, 32768, 65536, 98304)
CH_SIZE = (32768, 32768, 32768, 2048)
NQ = 4                 # SWDGE queues
CALLB = 8              # blocks per gather call (1024 idx = HW max)
GBUFS = 26             # gather call buffers in flight
PBUFS = 13             # P call buffers in flight

_cache = {}

bf16 = ml_dtypes.bfloat16

_ONEHOT = None


def _register_onehot():
    """Register the fused one-hot-times-scale DVE op:
    out[p, s, j] = (in0[p, s, j] == (k - (s0 + s*s1))) * in1[p, s, j]
    with k the global element index. With s0=0, s1=N (page width), the
    comparison target is the within-page index j."""
    global _ONEHOT
    if _ONEHOT is not None:
        return _ONEHOT
    import concourse.dve_ops as dve_ops
    from concourse.dve_ops import DveOp, OPS, _CUSTOM_DVE_ROW_BASE
    from concourse.dve_spec import Spec, Src0, Src1, C0, C1, eq, Idx, PageIdx, lower
    from concourse.dve_uop import DveOpSpec

    name = "ONEHOT_SVAL_ANT"
    if name in dve_ops._SUB_OPCODE_FOR_NAME:
        _ONEHOT = next(op for op in OPS if op.name == name)
        return _ONEHOT

    def _onehot_ref(in0, in1, s0, s1, imm2):
        P = in0.shape[0]
        Sd = int(np.prod(in0.shape[1:-1]))
        Nd = in0.shape[-1]
        slot = in0.reshape(P, Sd, Nd).astype(np.float32)
        sval = in1.reshape(P, Sd, Nd).astype(np.float32)
        k = np.arange(Sd * Nd, dtype=np.float32).reshape(1, Sd, Nd)
        s0v = s0[:, None] if isinstance(s0, np.ndarray) else s0
        s1v = float(s1.flat[0]) if isinstance(s1, np.ndarray) else s1
        pg = s0v + np.arange(Sd, dtype=np.float32)[None, :, None] * s1v
        return ((slot == (k - pg)) * sval).reshape(in1.shape)

    spec = Spec(body=eq(Src0, Idx - PageIdx(C0, C1)) * Src1, reference=_onehot_ref)
    row = _CUSTOM_DVE_ROW_BASE + len(OPS)
    shas = {}
    for ver in ("v3", "v4"):
        s = DveOpSpec(name=name, opcode=row, uops=lower(spec, ver=ver), rd1_en=True)
        shas[ver] = s.sha(ver)
    op = DveOp(name, spec, subdim=True, uops_sha=shas)
    OPS.append(op)
    dve_ops.CUSTOM_DVE_SPECS[name] = spec
    dve_ops._SUB_OPCODE_FOR_NAME[name] = row
    _ONEHOT = op
    return op


def _prep(input, adj_rows, adj_cols, adj_vals, weight, linear_weight, bias):
    f32 = np.float32

    lw = np.asarray(linear_weight, np.float64)
    e = np.exp(lw - lw.max())
    mix = (e / e.sum()).astype(f32)
    mix0 = float(mix[0])

    rows = np.asarray(adj_rows).reshape(-1)
    cols = np.asarray(adj_cols).reshape(-1)
    sval = (np.asarray(adj_vals, f32) * mix[1:, None]).reshape(-1)

    core = rows // RPC
    NSEG = NST * NCH * S

    per_core = []
    counts = np.zeros((NCORES, NSEG), np.int64)
    for k in range(NCORES):
        m = core == k
        r = (rows[m] - k * RPC).astype(np.int32)
        c = cols[m].astype(np.int32)
        v = sval[m]
        t = r >> 7
        ch = np.minimum(c >> 15, 3)
        st = t // S
        # segment order: st-major, then chunk, then tile-within-supertile
        seg = (st * NCH + ch) * S + (t - st * S)
        order = np.argsort(seg, kind="stable")
        seg = seg[order]
        counts[k] = np.bincount(seg, minlength=NSEG)
        per_core.append((r[order], c[order], ch[order], v[order], seg))

    B = np.maximum(np.ceil(counts.max(axis=0) / 128).astype(np.int64), 1)
    seg_start = np.concatenate([[0], np.cumsum(B * 128)])
    epad = int(seg_start[-1])
    nblk = epad // 128

    xin = np.zeros((NPAD, D), bf16)
    xin[:N] = np.asarray(input, f32).astype(bf16)
    xlocT_f = np.zeros((D, NPAD), f32)
    xlocT_f[:, :N] = np.asarray(input, f32).T

    wmat = np.asarray(weight, f32).astype(bf16)
    bias_b = np.asarray(bias, f32).astype(bf16)[None, :]
    ones_b = np.ones((1, D), bf16)
    ident = np.eye(D, dtype=bf16)

    in_maps = []
    for k in range(NCORES):
        r, c, ch, v, seg = per_core[k]
        ncnt = counts[k]
        within = np.arange(len(r)) - np.repeat(
            np.concatenate([[0], np.cumsum(ncnt)[:-1]]), ncnt
        )
        dest = seg_start[seg] + within

        cpad = np.zeros(epad, np.int32)
        spad = np.full(epad, -1.0, f32)
        vpad = np.zeros(epad, f32)
        cbase = np.asarray(CH_BASE, np.int32)[ch]
        cpad[dest] = c - cbase
        spad[dest] = (r & (TILEW - 1)).astype(f32)
        vpad[dest] = v

        gidx16 = cpad.reshape(-1, 16).T.astype(np.int16)
        gidx = np.broadcast_to(gidx16, (8, 16, epad // 16)).reshape(128, epad // 16).copy()

        in_maps.append({
            "xin": xin,
            "gidx": gidx,
            "slot": spad.reshape(nblk, 128).T.astype(bf16).copy(),  # [128, nblk]
            "sval": vpad.reshape(nblk, 128).T.astype(bf16).copy(),
            "xlocT": np.ascontiguousarray(
                (mix0 * xlocT_f[:, k * RPC:(k + 1) * RPC]).astype(bf16)
            ),
            "wmat": wmat,
            "biasb": bias_b,
            "onesb": ones_b,
            "ident": ident,
        })
    return in_maps, B.reshape(NST, NCH, S), mix0


def _build(B, mix0):
    import concourse.bass as bass
    import concourse.bacc as bacc
    import concourse.mybir as mybir
    import concourse.tile as tile

    onehot_op = _register_onehot()

    dt = mybir.dt
    alu = mybir.AluOpType
    nblk = int(B.sum())
    epad = nblk * 128

    nc = bacc.Bacc(None, num_swdge_queues=NQ)
    xin_d = nc.declare_dram_parameter("xin", [NPAD, D], dt.bfloat16, isOutput=False)
    gidx_d = nc.declare_dram_parameter("gidx", [128, epad // 16], dt.int16, isOutput=False)
    slot_d = nc.declare_dram_parameter("slot", [128, nblk], dt.bfloat16, isOutput=False)
    sval_d = nc.declare_dram_parameter("sval", [128, nblk], dt.bfloat16, isOutput=False)
    xlocT_d = nc.declare_dram_parameter("xlocT", [128, RPC], dt.bfloat16, isOutput=False)
    wmat_d = nc.declare_dram_parameter("wmat", [D, D], dt.bfloat16, isOutput=False)
    bias_d = nc.declare_dram_parameter("biasb", [1, D], dt.bfloat16, isOutput=False)
    ones_d = nc.declare_dram_parameter("onesb", [1, D], dt.bfloat16, isOutput=False)
    ident_d = nc.declare_dram_parameter("ident", [D, D], dt.bfloat16, isOutput=False)
    out_d = nc.declare_dram_parameter("out", [RPC, D], dt.float32, isOutput=True)
    rep_d = nc.declare_dram_parameter("rep", [RPC, D], dt.float32, isOutput=True)

    # static plan: per (st, c): first block index and count (sum over S tiles)
    blk0 = np.zeros((NST, NCH), np.int64)
    nbc = np.zeros((NST, NCH), np.int64)
    g = 0
    for st in range(NST):
        for c in range(NCH):
            blk0[st, c] = g
            nbc[st, c] = int(B[st, c].sum())
            g += nbc[st, c]

    # split each (st, c) group into <=CALLB-block calls (HW caps one
    # dma_gather at 1024 indices; full calls make full 16KB DMA packets);
    # greedy queue assignment by load
    calls = []  # (st, c, sub0, cb)
    for st in range(NST):
        for c in range(NCH):
            nb = int(nbc[st, c])
            s0 = 0
            while s0 < nb:
                cb = min(CALLB, nb - s0)
                calls.append((st, c, s0, cb))
                s0 += cb
    qload = [0] * NQ
    qassign = []
    for _, _, _, cb in calls:
        q = min(range(NQ), key=lambda i: qload[i])
        qassign.append(q)
        qload[q] += cb

    with tile.TileContext(nc) as tc:
        with (
            tc.tile_pool(name="const", bufs=1) as const,
            tc.tile_pool(name="adj", bufs=1) as adj,
            tc.tile_pool(name="gbuf", bufs=GBUFS) as gbuf,
            tc.tile_pool(name="pbuf", bufs=PBUFS) as pbuf,
            tc.tile_pool(name="rcb", bufs=64) as rcb,
            tc.tile_pool(name="rbuf", bufs=6) as rbuf,
            tc.tile_pool(name="ps_acc", bufs=4, space="PSUM") as ps_acc,
            tc.tile_pool(name="ps_out", bufs=2, space="PSUM") as ps_out,
            tc.tile_pool(name="ps_rep", bufs=2, space="PSUM") as ps_rep,
        ):
            wmat = const.tile([D, D], dt.bfloat16)
            biasb = const.tile([1, D], dt.bfloat16)
            onesb = const.tile([1, D], dt.bfloat16)
            ident = const.tile([D, D], dt.bfloat16)
            xlocT = const.tile([128, RPC], dt.bfloat16)
            gidx = adj.tile([128, epad // 16], dt.int16)
            slot = adj.tile([128, nblk], dt.bfloat16)
            sval = adj.tile([128, nblk], dt.bfloat16)

            nc.sync.dma_start(wmat[:], wmat_d[:])
            nc.sync.dma_start(biasb[:], bias_d[:])
            nc.sync.dma_start(onesb[:], ones_d[:])
            nc.sync.dma_start(ident[:], ident_d[:])
            nc.sync.dma_start(xlocT[:], xlocT_d[:])
            nc.sync.dma_start(gidx[:], gidx_d[:])
            nc.sync.dma_start(slot[:], slot_d[:])
            nc.sync.dma_start(sval[:], sval_d[:])

            # all SWDGE gather calls upfront (ring-buffered)
            call_gt = []
            for ci, (st, c, s0, cb) in enumerate(calls):
                b0 = int(blk0[st, c]) + s0
                gt = gbuf.tile([128, cb * 128], dt.bfloat16, tag="gt")
                gt3 = gt[:].rearrange("p (b e) -> p b e", e=128)
                nc.gpsimd.dma_gather(
                    out_ap=gt3,
                    in_ap=xin_d[CH_BASE[c]:CH_BASE[c] + CH_SIZE[c], :],
                    idxs_ap=gidx[:, b0 * 8:(b0 + cb) * 8],
                    num_idxs=cb * 128,
                    num_idxs_reg=cb * 128,
                    elem_size=D,
                    queue_num=qassign[ci],
                )
                call_gt.append(gt3)

            # per supertile: per-(tile, chunk) contiguous PSUM chains
            # (PE accumulation chains must not interleave), flushed into
            # an SBUF f32 supertile accumulator; per-call P-builds emitted
            # as the block stream crosses call boundaries
            ci = 0
            pt3_cur = None
            for st in range(NST):
                parts = [[None] * NCH for _ in range(S)]
                for c in range(NCH):
                    g0 = int(blk0[st, c])  # group's first global block
                    jg = 0                 # within-group block cursor
                    for q in range(S):
                        bq = int(B[st, c, q])
                        if bq == 0:
                            continue
                        acc = ps_acc.tile([128, TILEW], dt.float32)
                        for j in range(bq):
                            # advance to the call covering block g0+jg
                            while ci < len(calls):
                                cst, cc, cs0, ccb = calls[ci]
                                if (cst, cc) == (st, c) and cs0 <= jg < cs0 + ccb:
                                    break
                                ci += 1
                            cst, cc, cs0, ccb = calls[ci]
                            if pt3_cur is None or pt3_cur[0] != ci:
                                b0 = g0 + cs0
                                pt = pbuf.tile([128, ccb * TILEW], dt.bfloat16, tag="pt")
                                pt3 = pt[:].rearrange("p (b j) -> p b j", j=TILEW)
                                sl = slot[:, b0:b0 + ccb]
                                sv = sval[:, b0:b0 + ccb]
                                sl_ap = bass.AP(
                                    sl.tensor, sl.offset, [sl.ap[0], [1, ccb], [0, TILEW]])
                                sv_ap = bass.AP(
                                    sv.tensor, sv.offset, [sv.ap[0], [1, ccb], [0, TILEW]])
                                nc.vector._custom_dve(
                                    onehot_op, out=pt3, in0=sl_ap, in1=sv_ap,
                                    s0=0.0, s1=float(TILEW),
                                )
                                pt3_cur = (ci, pt3)
                            lj = jg - cs0
                            nc.tensor.matmul(
                                acc[:], call_gt[ci][:, lj, :], pt3_cur[1][:, lj, :],
                                start=(j == 0), stop=(j == bq - 1),
                            )
                            jg += 1
                        pc = rcb.tile([128, TILEW], dt.bfloat16, tag="pc")
                        nc.scalar.copy(pc[:], acc[:])
                        parts[q][c] = pc
                        if c == NCH - 1:
                            t = st * S + q
                            outp = ps_out.tile([D, D], dt.float32)
                            ops = [p for p in parts[q] if p is not None]
                            nc.tensor.matmul(
                                outp[:], xlocT[:, t * TILEW:(t + 1) * TILEW],
                                wmat[:], start=True, stop=False)
                            for p in ops:
                                nc.tensor.matmul(outp[:], p[:], wmat[:],
                                                 start=False, stop=False)
                            nc.tensor.matmul(outp[:], onesb[:], biasb[:],
                                             start=False, stop=True)
                            repp = ps_rep.tile([D, D], dt.float32)
                            nc.tensor.matmul(
                                repp[:], xlocT[:, t * TILEW:(t + 1) * TILEW],
                                ident[:], start=True, stop=False)
                            for pi, p in enumerate(ops):
                                nc.tensor.matmul(repp[:], p[:], ident[:],
                                                 start=False, stop=(pi == len(ops) - 1))
                            outs = rbuf.tile([D, D], dt.float32, tag="outs")
                            reps = rbuf.tile([D, D], dt.float32, tag="reps")
                            nc.scalar.copy(outs[:], outp[:])
                            nc.scalar.copy(reps[:], repp[:])
                            r0 = t * TILEW
                            nc.sync.dma_start(out_d[r0:r0 + TILEW, :], outs[:])
                            nc.sync.dma_start(rep_d[r0:r0 + TILEW, :], reps[:])



    nc.compile()
    return nc


def kernel(**inputs):
    from concourse.bass_utils import run_bass_kernel_spmd

    in_maps, B, mix0 = _prep(**inputs)
    key = (tuple(B.reshape(-1)), round(mix0, 9))
    if key not in _cache:
        _cache.clear()
        _cache[key] = _build(B, mix0)
    nc = _cache[key]

    res = run_bass_kernel_spmd(nc, in_maps, list(range(NCORES)))
    out = np.concatenate([np.asarray(res.results[k]["out"]) for k in range(NCORES)])
    rep = np.concatenate([np.asarray(res.results[k]["rep"]) for k in range(NCORES)])
    return out[:N].astype(np.float32), rep[:N].astype(np.float32)


# revision 23
# speedup vs baseline: 1.0157x; 1.0157x over previous
"""AFGNN layer (6-hop sparse message passing + softmax mix + dense proj) on
8 TRN2 NeuronCores — v6 (1.23ms, vs 1.96ms baseline).

Structure:
  - Edges sharded by dest row across 8 cores; per core sorted into
    (supertile of 14 x 128-row tiles, col-chunk, tile) segments, padded
    to 128-edge blocks (B = max over cores, shared SPMD program).
  - SWDGE dma_gather in 1024-idx calls (HW cap; full calls make full
    16KB per-engine DMA packets - smaller packets pay ~0.7us fixed).
    Gather is DESCRIPTOR-RATE limited (~2.4ns/desc aggregate; 512B
    descriptors cost the same as 256B), so padding is the main lever:
    128-row tiles keep it ~12%.
  - P one-hot scatter matrices built by a runtime-registered fused
    custom-DVE op: eq(slot, Idx - PageIdx(0, W)) * sval over
    [128, blocks, W] in ONE instruction per gather call.
  - Per-(tile, chunk) PSUM matmul chains (PE accumulation chains MUST
    NOT interleave - interleaved chains in one bank corrupt), flushed
    into an SBUF f32 supertile accumulator (DVE add), consumed by
    per-tile epilogues (proj + bias + PE transpose for row-major rep)
    that stream output DMAs throughout the kernel.
"""

import numpy as np
import ml_dtypes

N = 100000
NPAD = 100352          # 784 * 128
D = 128
NCORES = 8
RPC = NPAD // NCORES   # 12544 rows per core
TILEW = 128
NT = RPC // TILEW      # 98 row tiles per core
S = 14                 # tiles per supertile
NST = NT // S          # 7
NCH = 4
CH_BASE = (0, 32768, 65536, 98304)
CH_SIZE = (32768, 32768, 32768, 2048)
NQ = 4                 # SWDGE queues
CALLB = 8              # blocks per gather call (1024 idx = HW max)
GBUFS = 20             # gather call buffers in flight
PBUFS = 12             # P call buffers in flight

_cache = {}

bf16 = ml_dtypes.bfloat16

_ONEHOT = None


def _register_onehot():
    """Register the fused one-hot-times-scale DVE op:
    out[p, s, j] = (in0[p, s, j] == (k - (s0 + s*s1))) * in1[p, s, j]
    with k the global element index. With s0=0, s1=N (page width), the
    comparison target is the within-page index j."""
    global _ONEHOT
    if _ONEHOT is not None:
        return _ONEHOT
    import concourse.dve_ops as dve_ops
    from concourse.dve_ops import DveOp, OPS, _CUSTOM_DVE_ROW_BASE
    from concourse.dve_spec import Spec, Src0, Src1, C0, C1, eq, Idx, PageIdx, lower
    from concourse.dve_uop import DveOpSpec

    name = "ONEHOT_SVAL_ANT"
    if name in dve_ops._SUB_OPCODE_FOR_NAME:
        _ONEHOT = next(op for op in OPS if op.name == name)
        return _ONEHOT

    def _onehot_ref(in0, in1, s0, s1, imm2):
        P = in0.shape[0]
        Sd = int(np.prod(in0.shape[1:-1]))
        Nd = in0.shape[-1]
        slot = in0.reshape(P, Sd, Nd).astype(np.float32)
        sval = in1.reshape(P, Sd, Nd).astype(np.float32)
        k = np.arange(Sd * Nd, dtype=np.float32).reshape(1, Sd, Nd)
        s0v = s0[:, None] if isinstance(s0, np.ndarray) else s0
        s1v = float(s1.flat[0]) if isinstance(s1, np.ndarray) else s1
        pg = s0v + np.arange(Sd, dtype=np.float32)[None, :, None] * s1v
        return ((slot == (k - pg)) * sval).reshape(in1.shape)

    spec = Spec(body=eq(Src0, Idx - PageIdx(C0, C1)) * Src1, reference=_onehot_ref)
    row = _CUSTOM_DVE_ROW_BASE + len(OPS)
    shas = {}
    for ver in ("v3", "v4"):
        s = DveOpSpec(name=name, opcode=row, uops=lower(spec, ver=ver), rd1_en=True)
        shas[ver] = s.sha(ver)
    op = DveOp(name, spec, subdim=True, uops_sha=shas)
    OPS.append(op)
    dve_ops.CUSTOM_DVE_SPECS[name] = spec
    dve_ops._SUB_OPCODE_FOR_NAME[name] = row
    _ONEHOT = op
    return op


def _prep(input, adj_rows, adj_cols, adj_vals, weight, linear_weight, bias):
    f32 = np.float32

    lw = np.asarray(linear_weight, np.float64)
    e = np.exp(lw - lw.max())
    mix = (e / e.sum()).astype(f32)
    mix0 = float(mix[0])

    rows = np.asarray(adj_rows).reshape(-1)
    cols = np.asarray(adj_cols).reshape(-1)
    sval = (np.asarray(adj_vals, f32) * mix[1:, None]).reshape(-1)

    core = rows // RPC
    NSEG = NST * NCH * S

    per_core = []
    counts = np.zeros((NCORES, NSEG), np.int64)
    for k in range(NCORES):
        m = core == k
        r = (rows[m] - k * RPC).astype(np.int32)
        c = cols[m].astype(np.int32)
        v = sval[m]
        t = r >> 7
        ch = np.minimum(c >> 15, 3)
        st = t // S
        # segment order: st-major, then chunk, then tile-within-supertile
        seg = (st * NCH + ch) * S + (t - st * S)
        order = np.argsort(seg, kind="stable")
        seg = seg[order]
        counts[k] = np.bincount(seg, minlength=NSEG)
        per_core.append((r[order], c[order], ch[order], v[order], seg))

    B = np.maximum(np.ceil(counts.max(axis=0) / 128).astype(np.int64), 1)
    seg_start = np.concatenate([[0], np.cumsum(B * 128)])
    epad = int(seg_start[-1])
    nblk = epad // 128

    xin = np.zeros((NPAD, D), bf16)
    xin[:N] = np.asarray(input, f32).astype(bf16)
    xlocT_f = np.zeros((D, NPAD), f32)
    xlocT_f[:, :N] = np.asarray(input, f32).T

    wmat = np.asarray(weight, f32).astype(bf16)
    bias_b = np.asarray(bias, f32).astype(bf16)[None, :]
    ones_b = np.ones((1, D), bf16)
    ident = np.eye(D, dtype=bf16)

    in_maps = []
    for k in range(NCORES):
        r, c, ch, v, seg = per_core[k]
        ncnt = counts[k]
        within = np.arange(len(r)) - np.repeat(
            np.concatenate([[0], np.cumsum(ncnt)[:-1]]), ncnt
        )
        dest = seg_start[seg] + within

        cpad = np.zeros(epad, np.int32)
        spad = np.full(epad, -1.0, f32)
        vpad = np.zeros(epad, f32)
        cbase = np.asarray(CH_BASE, np.int32)[ch]
        cpad[dest] = c - cbase
        spad[dest] = (r & (TILEW - 1)).astype(f32)
        vpad[dest] = v

        gidx16 = cpad.reshape(-1, 16).T.astype(np.int16)
        gidx = np.broadcast_to(gidx16, (8, 16, epad // 16)).reshape(128, epad // 16).copy()

        in_maps.append({
            "xin": xin,
            "gidx": gidx,
            "slot": spad.reshape(nblk, 128).T.astype(bf16).copy(),  # [128, nblk]
            "sval": vpad.reshape(nblk, 128).T.astype(bf16).copy(),
            "xlocT": np.ascontiguousarray(
                xlocT_f[:, k * RPC:(k + 1) * RPC].astype(bf16)
            ),
            "wmat": wmat,
            "biasb": bias_b,
            "onesb": ones_b,
            "ident": ident,
        })
    return in_maps, B.reshape(NST, NCH, S), mix0


def _build(B, mix0):
    import concourse.bass as bass
    import concourse.bacc as bacc
    import concourse.mybir as mybir
    import concourse.tile as tile

    onehot_op = _register_onehot()

    dt = mybir.dt
    alu = mybir.AluOpType
    nblk = int(B.sum())
    epad = nblk * 128

    nc = bacc.Bacc(None, num_swdge_queues=NQ)
    xin_d = nc.declare_dram_parameter("xin", [NPAD, D], dt.bfloat16, isOutput=False)
    gidx_d = nc.declare_dram_parameter("gidx", [128, epad // 16], dt.int16, isOutput=False)
    slot_d = nc.declare_dram_parameter("slot", [128, nblk], dt.bfloat16, isOutput=False)
    sval_d = nc.declare_dram_parameter("sval", [128, nblk], dt.bfloat16, isOutput=False)
    xlocT_d = nc.declare_dram_parameter("xlocT", [128, RPC], dt.bfloat16, isOutput=False)
    wmat_d = nc.declare_dram_parameter("wmat", [D, D], dt.bfloat16, isOutput=False)
    bias_d = nc.declare_dram_parameter("biasb", [1, D], dt.bfloat16, isOutput=False)
    ones_d = nc.declare_dram_parameter("onesb", [1, D], dt.bfloat16, isOutput=False)
    ident_d = nc.declare_dram_parameter("ident", [D, D], dt.bfloat16, isOutput=False)
    out_d = nc.declare_dram_parameter("out", [RPC, D], dt.float32, isOutput=True)
    rep_d = nc.declare_dram_parameter("rep", [RPC, D], dt.float32, isOutput=True)

    # static plan: per (st, c): first block index and count (sum over S tiles)
    blk0 = np.zeros((NST, NCH), np.int64)
    nbc = np.zeros((NST, NCH), np.int64)
    g = 0
    for st in range(NST):
        for c in range(NCH):
            blk0[st, c] = g
            nbc[st, c] = int(B[st, c].sum())
            g += nbc[st, c]

    # split each (st, c) group into <=CALLB-block calls (HW caps one
    # dma_gather at 1024 indices; full calls make full 16KB DMA packets);
    # greedy queue assignment by load
    calls = []  # (st, c, sub0, cb)
    for st in range(NST):
        for c in range(NCH):
            nb = int(nbc[st, c])
            s0 = 0
            while s0 < nb:
                cb = min(CALLB, nb - s0)
                calls.append((st, c, s0, cb))
                s0 += cb
    qload = [0] * NQ
    qassign = []
    for _, _, _, cb in calls:
        q = min(range(NQ), key=lambda i: qload[i])
        qassign.append(q)
        qload[q] += cb

    with tile.TileContext(nc) as tc:
        with (
            tc.tile_pool(name="const", bufs=1) as const,
            tc.tile_pool(name="adj", bufs=1) as adj,
            tc.tile_pool(name="gbuf", bufs=GBUFS) as gbuf,
            tc.tile_pool(name="pbuf", bufs=PBUFS) as pbuf,
            tc.tile_pool(name="racc", bufs=2) as racc,
            tc.tile_pool(name="rbuf", bufs=6) as rbuf,
            tc.tile_pool(name="ps_acc", bufs=4, space="PSUM") as ps_acc,
            tc.tile_pool(name="ps_out", bufs=2, space="PSUM") as ps_out,
            tc.tile_pool(name="ps_rep", bufs=2, space="PSUM") as ps_rep,
        ):
            wmat = const.tile([D, D], dt.bfloat16)
            biasb = const.tile([1, D], dt.bfloat16)
            onesb = const.tile([1, D], dt.bfloat16)
            ident = const.tile([D, D], dt.bfloat16)
            xlocT = const.tile([128, RPC], dt.bfloat16)
            gidx = adj.tile([128, epad // 16], dt.int16)
            slot = adj.tile([128, nblk], dt.bfloat16)
            sval = adj.tile([128, nblk], dt.bfloat16)

            nc.sync.dma_start(wmat[:], wmat_d[:])
            nc.sync.dma_start(biasb[:], bias_d[:])
            nc.sync.dma_start(onesb[:], ones_d[:])
            nc.sync.dma_start(ident[:], ident_d[:])
            nc.sync.dma_start(xlocT[:], xlocT_d[:])
            nc.sync.dma_start(gidx[:], gidx_d[:])
            nc.sync.dma_start(slot[:], slot_d[:])
            nc.sync.dma_start(sval[:], sval_d[:])

            # all SWDGE gather calls upfront (ring-buffered)
            call_gt = []
            for ci, (st, c, s0, cb) in enumerate(calls):
                b0 = int(blk0[st, c]) + s0
                gt = gbuf.tile([128, cb * 128], dt.bfloat16, tag="gt")
                gt3 = gt[:].rearrange("p (b e) -> p b e", e=128)
                nc.gpsimd.dma_gather(
                    out_ap=gt3,
                    in_ap=xin_d[CH_BASE[c]:CH_BASE[c] + CH_SIZE[c], :],
                    idxs_ap=gidx[:, b0 * 8:(b0 + cb) * 8],
                    num_idxs=cb * 128,
                    num_idxs_reg=cb * 128,
                    elem_size=D,
                    queue_num=qassign[ci],
                )
                call_gt.append(gt3)

            # per supertile: per-(tile, chunk) contiguous PSUM chains
            # (PE accumulation chains must not interleave), flushed into
            # an SBUF f32 supertile accumulator; per-call P-builds emitted
            # as the block stream crosses call boundaries
            ci = 0
            pt3_cur = None
            for st in range(NST):
                rT = racc.tile([128, S * TILEW], dt.float32, tag="rT")
                for c in range(NCH):
                    g0 = int(blk0[st, c])  # group's first global block
                    jg = 0                 # within-group block cursor
                    for q in range(S):
                        bq = int(B[st, c, q])
                        if bq == 0:
                            continue
                        acc = ps_acc.tile([128, TILEW], dt.float32)
                        for j in range(bq):
                            # advance to the call covering block g0+jg
                            while ci < len(calls):
                                cst, cc, cs0, ccb = calls[ci]
                                if (cst, cc) == (st, c) and cs0 <= jg < cs0 + ccb:
                                    break
                                ci += 1
                            cst, cc, cs0, ccb = calls[ci]
                            if pt3_cur is None or pt3_cur[0] != ci:
                                b0 = g0 + cs0
                                pt = pbuf.tile([128, ccb * TILEW], dt.bfloat16, tag="pt")
                                pt3 = pt[:].rearrange("p (b j) -> p b j", j=TILEW)
                                sl = slot[:, b0:b0 + ccb]
                                sv = sval[:, b0:b0 + ccb]
                                sl_ap = bass.AP(
                                    sl.tensor, sl.offset, [sl.ap[0], [1, ccb], [0, TILEW]])
                                sv_ap = bass.AP(
                                    sv.tensor, sv.offset, [sv.ap[0], [1, ccb], [0, TILEW]])
                                nc.vector._custom_dve(
                                    onehot_op, out=pt3, in0=sl_ap, in1=sv_ap,
                                    s0=0.0, s1=float(TILEW),
                                )
                                pt3_cur = (ci, pt3)
                            lj = jg - cs0
                            nc.tensor.matmul(
                                acc[:], call_gt[ci][:, lj, :], pt3_cur[1][:, lj, :],
                                start=(j == 0), stop=(j == bq - 1),
                            )
                            jg += 1
                        rslc = rT[:, q * TILEW:(q + 1) * TILEW]
                        if c == 0:
                            nc.vector.tensor_copy(rslc, acc[:])
                        else:
                            nc.vector.tensor_add(rslc, rslc, acc[:])

                for q in range(S):
                    t = st * S + q
                    rbf = rbuf.tile([D, D], dt.bfloat16, tag="rbf")
                    nc.vector.scalar_tensor_tensor(
                        rbf[:], xlocT[:, t * TILEW:(t + 1) * TILEW], mix0,
                        rT[:, q * TILEW:(q + 1) * TILEW], alu.mult, alu.add,
                    )
                    outp = ps_out.tile([D, D], dt.float32)
                    nc.tensor.matmul(outp[:], rbf[:], wmat[:], start=True, stop=False)
                    nc.tensor.matmul(outp[:], onesb[:], biasb[:], start=False, stop=True)
                    repp = ps_rep.tile([D, D], dt.float32)
                    nc.tensor.matmul(repp[:], rbf[:], ident[:], start=True, stop=True)
                    outs = rbuf.tile([D, D], dt.float32, tag="outs")
                    reps = rbuf.tile([D, D], dt.float32, tag="reps")
                    nc.scalar.copy(outs[:], outp[:])
                    nc.scalar.copy(reps[:], repp[:])
                    r0 = t * TILEW
                    nc.sync.dma_start(out_d[r0:r0 + TILEW, :], outs[:])
                    nc.sync.dma_start(rep_d[r0:r0 + TILEW, :], reps[:])



    nc.compile()
    return nc


def kernel(**inputs):
    from concourse.bass_utils import run_bass_kernel_spmd

    in_maps, B, mix0 = _prep(**inputs)
    key = (tuple(B.reshape(-1)), round(mix0, 9))
    if key not in _cache:
        _cache.clear()
        _cache[key] = _build(B, mix0)
    nc = _cache[key]

    res = run_bass_kernel_spmd(nc, in_maps, list(range(NCORES)))
    out = np.concatenate([np.asarray(res.results[k]["out"]) for k in range(NCORES)])
    rep = np.concatenate([np.asarray(res.results[k]["rep"]) for k in range(NCORES)])
    return out[:N].astype(np.float32), rep[:N].astype(np.float32)


# revision 24
# speedup vs baseline: 1.0180x; 1.0022x over previous
"""AFGNN layer (6-hop sparse message passing + softmax mix + dense proj) on
8 TRN2 NeuronCores — v6 (1.23ms, vs 1.96ms baseline).

Structure:
  - Edges sharded by dest row across 8 cores; per core sorted into
    (supertile of 14 x 128-row tiles, col-chunk, tile) segments, padded
    to 128-edge blocks (B = max over cores, shared SPMD program).
  - SWDGE dma_gather in 1024-idx calls (HW cap; full calls make full
    16KB per-engine DMA packets - smaller packets pay ~0.7us fixed).
    Gather is DESCRIPTOR-RATE limited (~2.4ns/desc aggregate; 512B
    descriptors cost the same as 256B), so padding is the main lever:
    128-row tiles keep it ~12%.
  - P one-hot scatter matrices built by a runtime-registered fused
    custom-DVE op: eq(slot, Idx - PageIdx(0, W)) * sval over
    [128, blocks, W] in ONE instruction per gather call.
  - Per-(tile, chunk) PSUM matmul chains (PE accumulation chains MUST
    NOT interleave - interleaved chains in one bank corrupt), flushed
    into an SBUF f32 supertile accumulator (DVE add), consumed by
    per-tile epilogues (proj + bias + PE transpose for row-major rep)
    that stream output DMAs throughout the kernel.
"""

import numpy as np
import ml_dtypes

N = 100000
NPAD = 100352          # 784 * 128
D = 128
NCORES = 8
RPC = NPAD // NCORES   # 12544 rows per core
TILEW = 128
NT = RPC // TILEW      # 98 row tiles per core
S = 14                 # tiles per supertile
NST = NT // S          # 7
NCH = 4
CH_BASE = (0, 32768, 65536, 98304)
CH_SIZE = (32768, 32768, 32768, 2048)
NQ = 4                 # SWDGE queues
CALLB = 8              # blocks per gather call (1024 idx = HW max)
GBUFS = 20             # gather call buffers in flight
PBUFS = 12             # P call buffers in flight

_cache = {}

bf16 = ml_dtypes.bfloat16

_ONEHOT = None


def _register_onehot():
    """Register the fused one-hot-times-scale DVE op:
    out[p, s, j] = (in0[p, s, j] == (k - (s0 + s*s1))) * in1[p, s, j]
    with k the global element index. With s0=0, s1=N (page width), the
    comparison target is the within-page index j."""
    global _ONEHOT
    if _ONEHOT is not None:
        return _ONEHOT
    import concourse.dve_ops as dve_ops
    from concourse.dve_ops import DveOp, OPS, _CUSTOM_DVE_ROW_BASE
    from concourse.dve_spec import Spec, Src0, Src1, C0, C1, eq, Idx, PageIdx, lower
    from concourse.dve_uop import DveOpSpec

    name = "ONEHOT_SVAL_ANT"
    if name in dve_ops._SUB_OPCODE_FOR_NAME:
        _ONEHOT = next(op for op in OPS if op.name == name)
        return _ONEHOT

    def _onehot_ref(in0, in1, s0, s1, imm2):
        P = in0.shape[0]
        Sd = int(np.prod(in0.shape[1:-1]))
        Nd = in0.shape[-1]
        slot = in0.reshape(P, Sd, Nd).astype(np.float32)
        sval = in1.reshape(P, Sd, Nd).astype(np.float32)
        k = np.arange(Sd * Nd, dtype=np.float32).reshape(1, Sd, Nd)
        s0v = s0[:, None] if isinstance(s0, np.ndarray) else s0
        s1v = float(s1.flat[0]) if isinstance(s1, np.ndarray) else s1
        pg = s0v + np.arange(Sd, dtype=np.float32)[None, :, None] * s1v
        return ((slot == (k - pg)) * sval).reshape(in1.shape)

    spec = Spec(body=eq(Src0, Idx - PageIdx(C0, C1)) * Src1, reference=_onehot_ref)
    row = _CUSTOM_DVE_ROW_BASE + len(OPS)
    shas = {}
    for ver in ("v3", "v4"):
        s = DveOpSpec(name=name, opcode=row, uops=lower(spec, ver=ver), rd1_en=True)
        shas[ver] = s.sha(ver)
    op = DveOp(name, spec, subdim=True, uops_sha=shas)
    OPS.append(op)
    dve_ops.CUSTOM_DVE_SPECS[name] = spec
    dve_ops._SUB_OPCODE_FOR_NAME[name] = row
    _ONEHOT = op
    return op


def _prep(input, adj_rows, adj_cols, adj_vals, weight, linear_weight, bias):
    f32 = np.float32

    lw = np.asarray(linear_weight, np.float64)
    e = np.exp(lw - lw.max())
    mix = (e / e.sum()).astype(f32)
    mix0 = float(mix[0])

    rows = np.asarray(adj_rows).reshape(-1)
    cols = np.asarray(adj_cols).reshape(-1)
    sval = (np.asarray(adj_vals, f32) * mix[1:, None]).reshape(-1)

    core = rows // RPC
    NSEG = NST * NCH * S

    per_core = []
    counts = np.zeros((NCORES, NSEG), np.int64)
    for k in range(NCORES):
        m = core == k
        r = (rows[m] - k * RPC).astype(np.int32)
        c = cols[m].astype(np.int32)
        v = sval[m]
        t = r >> 7
        ch = np.minimum(c >> 15, 3)
        st = t // S
        # segment order: st-major, then chunk, then tile-within-supertile
        seg = (st * NCH + ch) * S + (t - st * S)
        order = np.argsort(seg, kind="stable")
        seg = seg[order]
        counts[k] = np.bincount(seg, minlength=NSEG)
        per_core.append((r[order], c[order], ch[order], v[order], seg))

    B = np.maximum(np.ceil(counts.max(axis=0) / 128).astype(np.int64), 1)
    seg_start = np.concatenate([[0], np.cumsum(B * 128)])
    epad = int(seg_start[-1])
    nblk = epad // 128

    xin = np.zeros((NPAD, D), bf16)
    xin[:N] = np.asarray(input, f32).astype(bf16)
    xlocT_f = np.zeros((D, NPAD), f32)
    xlocT_f[:, :N] = np.asarray(input, f32).T

    wmat = np.asarray(weight, f32).astype(bf16)
    bias_b = np.asarray(bias, f32).astype(bf16)[None, :]
    ones_b = np.ones((1, D), bf16)
    ident = np.eye(D, dtype=bf16)

    in_maps = []
    for k in range(NCORES):
        r, c, ch, v, seg = per_core[k]
        ncnt = counts[k]
        within = np.arange(len(r)) - np.repeat(
            np.concatenate([[0], np.cumsum(ncnt)[:-1]]), ncnt
        )
        dest = seg_start[seg] + within

        cpad = np.zeros(epad, np.int32)
        spad = np.full(epad, -1.0, f32)
        vpad = np.zeros(epad, f32)
        cbase = np.asarray(CH_BASE, np.int32)[ch]
        cpad[dest] = c - cbase
        spad[dest] = (r & (TILEW - 1)).astype(f32)
        vpad[dest] = v

        gidx16 = cpad.reshape(-1, 16).T.astype(np.int16)
        gidx = np.broadcast_to(gidx16, (8, 16, epad // 16)).reshape(128, epad // 16).copy()

        in_maps.append({
            "xin": xin,
            "gidx": gidx,
            "slot": spad.reshape(nblk, 128).T.astype(bf16).copy(),  # [128, nblk]
            "sval": vpad.reshape(nblk, 128).T.astype(bf16).copy(),
            "xlocT": np.ascontiguousarray(
                xlocT_f[:, k * RPC:(k + 1) * RPC].astype(bf16)
            ),
            "wmat": wmat,
            "biasb": bias_b,
            "onesb": ones_b,
            "ident": ident,
        })
    return in_maps, B.reshape(NST, NCH, S), mix0


def _build(B, mix0):
    import concourse.bass as bass
    import concourse.bacc as bacc
    import concourse.mybir as mybir
    import concourse.tile as tile

    onehot_op = _register_onehot()

    dt = mybir.dt
    alu = mybir.AluOpType
    nblk = int(B.sum())
    epad = nblk * 128

    nc = bacc.Bacc(None, num_swdge_queues=NQ)
    xin_d = nc.declare_dram_parameter("xin", [NPAD, D], dt.bfloat16, isOutput=False)
    gidx_d = nc.declare_dram_parameter("gidx", [128, epad // 16], dt.int16, isOutput=False)
    slot_d = nc.declare_dram_parameter("slot", [128, nblk], dt.bfloat16, isOutput=False)
    sval_d = nc.declare_dram_parameter("sval", [128, nblk], dt.bfloat16, isOutput=False)
    xlocT_d = nc.declare_dram_parameter("xlocT", [128, RPC], dt.bfloat16, isOutput=False)
    wmat_d = nc.declare_dram_parameter("wmat", [D, D], dt.bfloat16, isOutput=False)
    bias_d = nc.declare_dram_parameter("biasb", [1, D], dt.bfloat16, isOutput=False)
    ones_d = nc.declare_dram_parameter("onesb", [1, D], dt.bfloat16, isOutput=False)
    ident_d = nc.declare_dram_parameter("ident", [D, D], dt.bfloat16, isOutput=False)
    out_d = nc.declare_dram_parameter("out", [RPC, D], dt.float32, isOutput=True)
    rep_d = nc.declare_dram_parameter("rep", [RPC, D], dt.float32, isOutput=True)

    # static plan: per (st, c): first block index and count (sum over S tiles)
    blk0 = np.zeros((NST, NCH), np.int64)
    nbc = np.zeros((NST, NCH), np.int64)
    g = 0
    for st in range(NST):
        for c in range(NCH):
            blk0[st, c] = g
            nbc[st, c] = int(B[st, c].sum())
            g += nbc[st, c]

    # split each (st, c) group into <=CALLB-block calls (HW caps one
    # dma_gather at 1024 indices; full calls make full 16KB DMA packets);
    # greedy queue assignment by load
    calls = []  # (st, c, sub0, cb)
    for st in range(NST):
        for c in range(NCH):
            nb = int(nbc[st, c])
            s0 = 0
            while s0 < nb:
                cb = min(CALLB, nb - s0)
                calls.append((st, c, s0, cb))
                s0 += cb
    qload = [0] * NQ
    qassign = []
    for _, _, _, cb in calls:
        q = min(range(NQ), key=lambda i: qload[i])
        qassign.append(q)
        qload[q] += cb

    with tile.TileContext(nc) as tc:
        with (
            tc.tile_pool(name="const", bufs=1) as const,
            tc.tile_pool(name="adj", bufs=1) as adj,
            tc.tile_pool(name="gbuf", bufs=GBUFS) as gbuf,
            tc.tile_pool(name="pbuf", bufs=PBUFS) as pbuf,
            tc.tile_pool(name="racc", bufs=2) as racc,
            tc.tile_pool(name="rbuf", bufs=6) as rbuf,
            tc.tile_pool(name="ps_acc", bufs=4, space="PSUM") as ps_acc,
            tc.tile_pool(name="ps_out", bufs=2, space="PSUM") as ps_out,
            tc.tile_pool(name="ps_rep", bufs=2, space="PSUM") as ps_rep,
        ):
            wmat = const.tile([D, D], dt.bfloat16)
            biasb = const.tile([1, D], dt.bfloat16)
            onesb = const.tile([1, D], dt.bfloat16)
            ident = const.tile([D, D], dt.bfloat16)
            xlocT = const.tile([128, RPC], dt.bfloat16)
            gidx = adj.tile([128, epad // 16], dt.int16)
            slot = adj.tile([128, nblk], dt.bfloat16)
            sval = adj.tile([128, nblk], dt.bfloat16)

            nc.sync.dma_start(wmat[:], wmat_d[:])
            nc.sync.dma_start(biasb[:], bias_d[:])
            nc.sync.dma_start(onesb[:], ones_d[:])
            nc.sync.dma_start(ident[:], ident_d[:])
            nc.sync.dma_start(xlocT[:], xlocT_d[:])
            # split index loads per supertile so the first gather calls
            # depend only on the first slice, not the whole 8MB load
            for st in range(NST):
                b0 = int(blk0[st, 0])
                b1 = int(blk0[st + 1, 0]) if st + 1 < NST else nblk
                nc.sync.dma_start(gidx[:, b0 * 8:b1 * 8], gidx_d[:, b0 * 8:b1 * 8])
                nc.sync.dma_start(slot[:, b0:b1], slot_d[:, b0:b1])
                nc.sync.dma_start(sval[:, b0:b1], sval_d[:, b0:b1])

            # all SWDGE gather calls upfront (ring-buffered)
            call_gt = []
            for ci, (st, c, s0, cb) in enumerate(calls):
                b0 = int(blk0[st, c]) + s0
                gt = gbuf.tile([128, cb * 128], dt.bfloat16, tag="gt")
                gt3 = gt[:].rearrange("p (b e) -> p b e", e=128)
                nc.gpsimd.dma_gather(
                    out_ap=gt3,
                    in_ap=xin_d[CH_BASE[c]:CH_BASE[c] + CH_SIZE[c], :],
                    idxs_ap=gidx[:, b0 * 8:(b0 + cb) * 8],
                    num_idxs=cb * 128,
                    num_idxs_reg=cb * 128,
                    elem_size=D,
                    queue_num=qassign[ci],
                )
                call_gt.append(gt3)

            # per supertile: per-(tile, chunk) contiguous PSUM chains
            # (PE accumulation chains must not interleave), flushed into
            # an SBUF f32 supertile accumulator; per-call P-builds emitted
            # as the block stream crosses call boundaries
            ci = 0
            pt3_cur = None
            for st in range(NST):
                rT = racc.tile([128, S * TILEW], dt.float32, tag="rT")
                for c in range(NCH):
                    g0 = int(blk0[st, c])  # group's first global block
                    jg = 0                 # within-group block cursor
                    for q in range(S):
                        bq = int(B[st, c, q])
                        if bq == 0:
                            continue
                        acc = ps_acc.tile([128, TILEW], dt.float32)
                        for j in range(bq):
                            # advance to the call covering block g0+jg
                            while ci < len(calls):
                                cst, cc, cs0, ccb = calls[ci]
                                if (cst, cc) == (st, c) and cs0 <= jg < cs0 + ccb:
                                    break
                                ci += 1
                            cst, cc, cs0, ccb = calls[ci]
                            if pt3_cur is None or pt3_cur[0] != ci:
                                b0 = g0 + cs0
                                pt = pbuf.tile([128, ccb * TILEW], dt.bfloat16, tag="pt")
                                pt3 = pt[:].rearrange("p (b j) -> p b j", j=TILEW)
                                sl = slot[:, b0:b0 + ccb]
                                sv = sval[:, b0:b0 + ccb]
                                sl_ap = bass.AP(
                                    sl.tensor, sl.offset, [sl.ap[0], [1, ccb], [0, TILEW]])
                                sv_ap = bass.AP(
                                    sv.tensor, sv.offset, [sv.ap[0], [1, ccb], [0, TILEW]])
                                nc.vector._custom_dve(
                                    onehot_op, out=pt3, in0=sl_ap, in1=sv_ap,
                                    s0=0.0, s1=float(TILEW),
                                )
                                pt3_cur = (ci, pt3)
                            lj = jg - cs0
                            nc.tensor.matmul(
                                acc[:], call_gt[ci][:, lj, :], pt3_cur[1][:, lj, :],
                                start=(j == 0), stop=(j == bq - 1),
                            )
                            jg += 1
                        rslc = rT[:, q * TILEW:(q + 1) * TILEW]
                        if c == 0:
                            nc.vector.tensor_copy(rslc, acc[:])
                        else:
                            nc.vector.tensor_add(rslc, rslc, acc[:])

                for q in range(S):
                    t = st * S + q
                    rbf = rbuf.tile([D, D], dt.bfloat16, tag="rbf")
                    nc.vector.scalar_tensor_tensor(
                        rbf[:], xlocT[:, t * TILEW:(t + 1) * TILEW], mix0,
                        rT[:, q * TILEW:(q + 1) * TILEW], alu.mult, alu.add,
                    )
                    outp = ps_out.tile([D, D], dt.float32)
                    nc.tensor.matmul(outp[:], rbf[:], wmat[:], start=True, stop=False)
                    nc.tensor.matmul(outp[:], onesb[:], biasb[:], start=False, stop=True)
                    repp = ps_rep.tile([D, D], dt.float32)
                    nc.tensor.matmul(repp[:], rbf[:], ident[:], start=True, stop=True)
                    outs = rbuf.tile([D, D], dt.float32, tag="outs")
                    reps = rbuf.tile([D, D], dt.float32, tag="reps")
                    nc.scalar.copy(outs[:], outp[:])
                    nc.scalar.copy(reps[:], repp[:])
                    r0 = t * TILEW
                    nc.sync.dma_start(out_d[r0:r0 + TILEW, :], outs[:])
                    nc.sync.dma_start(rep_d[r0:r0 + TILEW, :], reps[:])



    nc.compile()
    return nc


def kernel(**inputs):
    from concourse.bass_utils import run_bass_kernel_spmd

    in_maps, B, mix0 = _prep(**inputs)
    key = (tuple(B.reshape(-1)), round(mix0, 9))
    if key not in _cache:
        _cache.clear()
        _cache[key] = _build(B, mix0)
    nc = _cache[key]

    res = run_bass_kernel_spmd(nc, in_maps, list(range(NCORES)))
    out = np.concatenate([np.asarray(res.results[k]["out"]) for k in range(NCORES)])
    rep = np.concatenate([np.asarray(res.results[k]["rep"]) for k in range(NCORES)])
    return out[:N].astype(np.float32), rep[:N].astype(np.float32)


# revision 32
# speedup vs baseline: 1.1185x; 1.0988x over previous
"""AFGNN layer (6-hop sparse message passing + softmax mix + dense proj) on
8 TRN2 NeuronCores — v6 (1.23ms, vs 1.96ms baseline).

Structure:
  - Edges sharded by dest row across 8 cores; per core sorted into
    (supertile of 14 x 128-row tiles, col-chunk, tile) segments, padded
    to 128-edge blocks (B = max over cores, shared SPMD program).
  - SWDGE dma_gather in 1024-idx calls (HW cap; full calls make full
    16KB per-engine DMA packets - smaller packets pay ~0.7us fixed).
    Gather is DESCRIPTOR-RATE limited (~2.4ns/desc aggregate; 512B
    descriptors cost the same as 256B), so padding is the main lever:
    128-row tiles keep it ~12%.
  - P one-hot scatter matrices built by a runtime-registered fused
    custom-DVE op: eq(slot, Idx - PageIdx(0, W)) * sval over
    [128, blocks, W] in ONE instruction per gather call.
  - Per-(tile, chunk) PSUM matmul chains (PE accumulation chains MUST
    NOT interleave - interleaved chains in one bank corrupt), flushed
    into an SBUF f32 supertile accumulator (DVE add), consumed by
    per-tile epilogues (proj + bias + PE transpose for row-major rep)
    that stream output DMAs throughout the kernel.
"""

import numpy as np
import ml_dtypes

N = 100000
NPAD = 100352          # 784 * 128
D = 128
NCORES = 8
RPC = NPAD // NCORES   # 12544 rows per core
TILEW = 128
NT = RPC // TILEW      # 98 row tiles per core
S = 14                 # tiles per supertile
NST = NT // S          # 7
NCH = 4
CH_BASE = (0, 32768, 65536, 98304)
CH_SIZE = (32768, 32768, 32768, 2048)
NQ = 4                 # SWDGE queues
CALLB = 8              # blocks per gather call (1024 idx = HW max)
GBUFS = 18             # gather call buffers in flight
PBUFS = 8              # P call buffers in flight

_cache = {}

bf16 = ml_dtypes.bfloat16

_ONEHOT = None


def _register_onehot():
    """Register the fused one-hot-times-scale DVE op:
    out[p, s, j] = (in0[p, s, j] == (k - (s0 + s*s1))) * in1[p, s, j]
    with k the global element index. With s0=0, s1=N (page width), the
    comparison target is the within-page index j."""
    global _ONEHOT
    if _ONEHOT is not None:
        return _ONEHOT
    import concourse.dve_ops as dve_ops
    from concourse.dve_ops import DveOp, OPS, _CUSTOM_DVE_ROW_BASE
    from concourse.dve_spec import Spec, Src0, Src1, C0, C1, eq, Idx, PageIdx, lower
    from concourse.dve_uop import DveOpSpec

    name = "ONEHOT_SVAL_ANT"
    if name in dve_ops._SUB_OPCODE_FOR_NAME:
        _ONEHOT = next(op for op in OPS if op.name == name)
        return _ONEHOT

    def _onehot_ref(in0, in1, s0, s1, imm2):
        P = in0.shape[0]
        Sd = int(np.prod(in0.shape[1:-1]))
        Nd = in0.shape[-1]
        slot = in0.reshape(P, Sd, Nd).astype(np.float32)
        sval = in1.reshape(P, Sd, Nd).astype(np.float32)
        k = np.arange(Sd * Nd, dtype=np.float32).reshape(1, Sd, Nd)
        s0v = s0[:, None] if isinstance(s0, np.ndarray) else s0
        s1v = float(s1.flat[0]) if isinstance(s1, np.ndarray) else s1
        pg = s0v + np.arange(Sd, dtype=np.float32)[None, :, None] * s1v
        return ((slot == (k - pg)) * sval).reshape(in1.shape)

    spec = Spec(body=eq(Src0, Idx - PageIdx(C0, C1)) * Src1, reference=_onehot_ref)
    row = _CUSTOM_DVE_ROW_BASE + len(OPS)
    shas = {}
    for ver in ("v3", "v4"):
        s = DveOpSpec(name=name, opcode=row, uops=lower(spec, ver=ver), rd1_en=True)
        shas[ver] = s.sha(ver)
    op = DveOp(name, spec, subdim=True, uops_sha=shas)
    OPS.append(op)
    dve_ops.CUSTOM_DVE_SPECS[name] = spec
    dve_ops._SUB_OPCODE_FOR_NAME[name] = row
    _ONEHOT = op
    return op


def _prep(input, adj_rows, adj_cols, adj_vals, weight, linear_weight, bias):
    f32 = np.float32

    lw = np.asarray(linear_weight, np.float64)
    e = np.exp(lw - lw.max())
    mix = (e / e.sum()).astype(f32)
    mix0 = float(mix[0])

    rows = np.asarray(adj_rows).reshape(-1)
    cols = np.asarray(adj_cols).reshape(-1)
    sval = (np.asarray(adj_vals, f32) * mix[1:, None]).reshape(-1)

    core = rows // RPC
    NGRP = NST * NCH

    per_core = []
    gcounts = np.zeros((NCORES, NGRP), np.int64)
    for k in range(NCORES):
        m = core == k
        r = (rows[m] - k * RPC).astype(np.int32)
        c = cols[m].astype(np.int32)
        v = sval[m]
        t = r >> 7
        ch = np.minimum(c >> 15, 3)
        st = t // S
        grp = st * NCH + ch
        order = np.lexsort((t, grp))   # by group, then tile within group
        gs = grp[order]
        gcounts[k] = np.bincount(gs, minlength=NGRP)
        per_core.append((r[order], c[order], ch[order], v[order], gs, t[order]))

    Bg = np.maximum(np.ceil(gcounts.max(axis=0) / 128).astype(np.int64), 1)
    gblk0 = np.concatenate([[0], np.cumsum(Bg)])   # group -> first global block
    nblk = int(Bg.sum())
    epad = nblk * 128

    # pair plan: per global block, the union tile range over cores
    tlo = np.full(nblk, 10 ** 9, np.int64)
    thi = np.full(nblk, -1, np.int64)
    for k in range(NCORES):
        r, c, ch, v, gs, ts = per_core[k]
        ncnt = gcounts[k]
        pos = np.arange(len(r)) - np.repeat(
            np.concatenate([[0], np.cumsum(ncnt)[:-1]]), ncnt)
        gblk = gblk0[gs] + (pos >> 7)
        np.minimum.at(tlo, gblk, ts)
        np.maximum.at(thi, gblk, ts)
    # guard: blocks never touched (shouldn't happen) -> tile of their group
    for g in range(NGRP):
        st = g // NCH
        for b in range(int(gblk0[g]), int(gblk0[g + 1])):
            if thi[b] < 0:
                tlo[b] = thi[b] = st * S
    npair_blk = thi - tlo + 1
    pair0 = np.concatenate([[0], np.cumsum(npair_blk)])  # block -> first pair
    npairs = int(pair0[-1])

    xin = np.zeros((NPAD, D), bf16)
    xin[:N] = np.asarray(input, f32).astype(bf16)
    xlocT_f = np.zeros((D, NPAD), f32)
    xlocT_f[:, :N] = np.asarray(input, f32).T

    wmat = np.asarray(weight, f32).astype(bf16)
    bias_b = np.asarray(bias, f32).astype(bf16)[None, :]
    ones_b = np.ones((1, D), bf16)
    ident = np.eye(D, dtype=bf16)

    in_maps = []
    for k in range(NCORES):
        r, c, ch, v, gs, ts = per_core[k]
        ncnt = gcounts[k]
        pos = np.arange(len(r)) - np.repeat(
            np.concatenate([[0], np.cumsum(ncnt)[:-1]]), ncnt)
        gblk = gblk0[gs] + (pos >> 7)
        lane = pos & 127

        cpad = np.zeros(epad, np.int32)
        cbase = np.asarray(CH_BASE, np.int32)[ch]
        cpad[gblk * 128 + lane] = c - cbase

        pidx = pair0[gblk] + (ts - tlo[gblk])
        pslot = np.full(npairs * 128, -1.0, f32)
        psval = np.zeros(npairs * 128, f32)
        pslot[pidx * 128 + lane] = (r & (TILEW - 1)).astype(f32)
        psval[pidx * 128 + lane] = v

        gidx16 = cpad.reshape(-1, 16).T.astype(np.int16)
        gidx = np.broadcast_to(gidx16, (8, 16, epad // 16)).reshape(128, epad // 16).copy()

        in_maps.append({
            "xin": xin,
            "gidx": gidx,
            "slot": pslot.reshape(npairs, 128).T.astype(bf16).copy(),  # [128, npairs]
            "sval": psval.reshape(npairs, 128).T.astype(bf16).copy(),
            "xlocT": np.ascontiguousarray(
                xlocT_f[:, k * RPC:(k + 1) * RPC].astype(bf16)
            ),
            "wmat": wmat,
            "biasb": bias_b,
            "onesb": ones_b,
            "ident": ident,
        })
    plan = (Bg, tlo, thi)
    return in_maps, plan, mix0


def _build(plan, mix0):
    import concourse.bass as bass
    import concourse.bacc as bacc
    import concourse.mybir as mybir
    import concourse.tile as tile

    onehot_op = _register_onehot()
    Bg, tlo, thi = plan

    dt = mybir.dt
    alu = mybir.AluOpType
    nblk = int(Bg.sum())
    epad = nblk * 128
    gblk0 = np.concatenate([[0], np.cumsum(Bg)])
    pair0 = np.concatenate([[0], np.cumsum(thi - tlo + 1)])
    npairs = int(pair0[-1])

    nc = bacc.Bacc(None, num_swdge_queues=NQ)
    xin_d = nc.declare_dram_parameter("xin", [NPAD, D], dt.bfloat16, isOutput=False)
    gidx_d = nc.declare_dram_parameter("gidx", [128, epad // 16], dt.int16, isOutput=False)
    slot_d = nc.declare_dram_parameter("slot", [128, npairs], dt.bfloat16, isOutput=False)
    sval_d = nc.declare_dram_parameter("sval", [128, npairs], dt.bfloat16, isOutput=False)
    xlocT_d = nc.declare_dram_parameter("xlocT", [128, RPC], dt.bfloat16, isOutput=False)
    wmat_d = nc.declare_dram_parameter("wmat", [D, D], dt.bfloat16, isOutput=False)
    bias_d = nc.declare_dram_parameter("biasb", [1, D], dt.bfloat16, isOutput=False)
    ones_d = nc.declare_dram_parameter("onesb", [1, D], dt.bfloat16, isOutput=False)
    ident_d = nc.declare_dram_parameter("ident", [D, D], dt.bfloat16, isOutput=False)
    out_d = nc.declare_dram_parameter("out", [RPC, D], dt.float32, isOutput=True)
    rep_d = nc.declare_dram_parameter("rep", [RPC, D], dt.float32, isOutput=True)

    # calls: per (st, c) group, slices of <=CALLB blocks
    calls = []  # (g, block0_global, cb)
    for g in range(NST * NCH):
        b0, b1 = int(gblk0[g]), int(gblk0[g + 1])
        # c3 blocks straddle many tiles (many pairs): smaller calls keep
        # the worst-case P buffer bounded
        cbmax = 4 if g % NCH == NCH - 1 else CALLB
        s0 = b0
        while s0 < b1:
            cb = min(cbmax, b1 - s0)
            calls.append((g, s0, cb))
            s0 += cb
    qload = [0] * NQ
    qassign = []
    for _, _, cb in calls:
        q = min(range(NQ), key=lambda i: qload[i])
        qassign.append(q)
        qload[q] += cb

    with tile.TileContext(nc) as tc:
        with (
            tc.tile_pool(name="const", bufs=1) as const,
            tc.tile_pool(name="adj", bufs=1) as adj,
            tc.tile_pool(name="gbuf", bufs=GBUFS) as gbuf,
            tc.tile_pool(name="pbuf", bufs=PBUFS) as pbuf,
            tc.tile_pool(name="racc", bufs=2) as racc,
            tc.tile_pool(name="rbuf", bufs=6) as rbuf,
            tc.tile_pool(name="ps_acc", bufs=4, space="PSUM") as ps_acc,
            tc.tile_pool(name="ps_out", bufs=2, space="PSUM") as ps_out,
            tc.tile_pool(name="ps_rep", bufs=2, space="PSUM") as ps_rep,
        ):
            wmat = const.tile([D, D], dt.bfloat16)
            biasb = const.tile([1, D], dt.bfloat16)
            onesb = const.tile([1, D], dt.bfloat16)
            ident = const.tile([D, D], dt.bfloat16)
            xlocT = const.tile([128, RPC], dt.bfloat16)
            gidx = adj.tile([128, epad // 16], dt.int16)
            slot = adj.tile([128, npairs], dt.bfloat16)
            sval = adj.tile([128, npairs], dt.bfloat16)

            nc.sync.dma_start(wmat[:], wmat_d[:])
            nc.sync.dma_start(biasb[:], bias_d[:])
            nc.sync.dma_start(onesb[:], ones_d[:])
            nc.sync.dma_start(ident[:], ident_d[:])
            nc.sync.dma_start(xlocT[:], xlocT_d[:])
            # split index loads per supertile so the first gather calls
            # depend only on the first slice, not the whole load
            for st in range(NST):
                b0 = int(gblk0[st * NCH])
                b1 = int(gblk0[min((st + 1) * NCH, NST * NCH)])
                p0, p1 = int(pair0[b0]), int(pair0[b1])
                nc.sync.dma_start(gidx[:, b0 * 8:b1 * 8], gidx_d[:, b0 * 8:b1 * 8])
                nc.sync.dma_start(slot[:, p0:p1], slot_d[:, p0:p1])
                nc.sync.dma_start(sval[:, p0:p1], sval_d[:, p0:p1])

            # all SWDGE gather calls upfront (ring-buffered)
            call_gt = []
            for ci, (g, b0, cb) in enumerate(calls):
                c = g % NCH
                gt = gbuf.tile([128, cb * 128], dt.bfloat16, tag="gt")
                gt3 = gt[:].rearrange("p (b e) -> p b e", e=128)
                nc.gpsimd.dma_gather(
                    out_ap=gt3,
                    in_ap=xin_d[CH_BASE[c]:CH_BASE[c] + CH_SIZE[c], :],
                    idxs_ap=gidx[:, b0 * 8:(b0 + cb) * 8],
                    num_idxs=cb * 128,
                    num_idxs_reg=cb * 128,
                    elem_size=D,
                    queue_num=qassign[ci],
                )
                call_gt.append(gt3)

            # per supertile: per-(tile, chunk) contiguous PSUM chains over
            # the pairs (block, tile) covering that tile, flushed into an
            # SBUF f32 supertile accumulator; per-call P-builds (one fused
            # DVE op over the call's PAIRS) emitted at first use
            blk2call = {}
            for ci, (g, b0, cb) in enumerate(calls):
                for b in range(b0, b0 + cb):
                    blk2call[b] = ci
            ptbuilt = {}

            def get_pt(ci):
                if ci not in ptbuilt:
                    cg, cb0, ccb = calls[ci]
                    pp0 = int(pair0[cb0])
                    pp1 = int(pair0[cb0 + ccb])
                    npc = pp1 - pp0
                    pt = pbuf.tile([128, npc * TILEW], dt.bfloat16, tag="pt")
                    pt3 = pt[:].rearrange("p (b j) -> p b j", j=TILEW)
                    sl = slot[:, pp0:pp1]
                    sv = sval[:, pp0:pp1]
                    sl_ap = bass.AP(
                        sl.tensor, sl.offset, [sl.ap[0], [1, npc], [0, TILEW]])
                    sv_ap = bass.AP(
                        sv.tensor, sv.offset, [sv.ap[0], [1, npc], [0, TILEW]])
                    nc.vector._custom_dve(
                        onehot_op, out=pt3, in0=sl_ap, in1=sv_ap,
                        s0=0.0, s1=float(TILEW),
                    )
                    ptbuilt[ci] = (pt3, pp0)
                return ptbuilt[ci]

            for st in range(NST):
                rT = racc.tile([128, S * TILEW], dt.float32, tag="rT")
                for c in range(NCH):
                    g = st * NCH + c
                    b0g, b1g = int(gblk0[g]), int(gblk0[g + 1])
                    for q in range(S):
                        t = st * S + q
                        blks = [b for b in range(b0g, b1g) if tlo[b] <= t <= thi[b]]
                        if not blks:
                            continue
                        acc = ps_acc.tile([128, TILEW], dt.float32)
                        for j, b in enumerate(blks):
                            ci = blk2call[b]
                            cg, cb0, ccb = calls[ci]
                            pt3, pp0 = get_pt(ci)
                            p = int(pair0[b]) + (t - int(tlo[b]))
                            nc.tensor.matmul(
                                acc[:], call_gt[ci][:, b - cb0, :],
                                pt3[:, p - pp0, :],
                                start=(j == 0), stop=(j == len(blks) - 1),
                            )
                        rslc = rT[:, q * TILEW:(q + 1) * TILEW]
                        if c == 0:
                            nc.vector.tensor_copy(rslc, acc[:])
                        else:
                            nc.vector.tensor_add(rslc, rslc, acc[:])

                for q in range(S):
                    t = st * S + q
                    rbf = rbuf.tile([D, D], dt.bfloat16, tag="rbf")
                    nc.vector.scalar_tensor_tensor(
                        rbf[:], xlocT[:, t * TILEW:(t + 1) * TILEW], mix0,
                        rT[:, q * TILEW:(q + 1) * TILEW], alu.mult, alu.add,
                    )
                    outp = ps_out.tile([D, D], dt.float32)
                    nc.tensor.matmul(outp[:], rbf[:], wmat[:], start=True, stop=False)
                    nc.tensor.matmul(outp[:], onesb[:], biasb[:], start=False, stop=True)
                    repp = ps_rep.tile([D, D], dt.float32)
                    nc.tensor.matmul(repp[:], rbf[:], ident[:], start=True, stop=True)
                    outs = rbuf.tile([D, D], dt.float32, tag="outs")
                    reps = rbuf.tile([D, D], dt.float32, tag="reps")
                    nc.scalar.copy(outs[:], outp[:])
                    nc.scalar.copy(reps[:], repp[:])
                    r0 = t * TILEW
                    nc.sync.dma_start(out_d[r0:r0 + TILEW, :], outs[:])
                    nc.sync.dma_start(rep_d[r0:r0 + TILEW, :], reps[:])

    nc.compile()
    return nc


def kernel(**inputs):
    from concourse.bass_utils import run_bass_kernel_spmd

    in_maps, plan, mix0 = _prep(**inputs)
    key = (tuple(plan[0]), tuple(plan[1]), tuple(plan[2]), round(mix0, 9))
    if key not in _cache:
        _cache.clear()
        _cache[key] = _build(plan, mix0)
    nc = _cache[key]

    res = run_bass_kernel_spmd(nc, in_maps, list(range(NCORES)))
    out = np.concatenate([np.asarray(res.results[k]["out"]) for k in range(NCORES)])
    rep = np.concatenate([np.asarray(res.results[k]["rep"]) for k in range(NCORES)])
    return out[:N].astype(np.float32), rep[:N].astype(np.float32)


# revision 34
# speedup vs baseline: 1.1275x; 1.0080x over previous
"""AFGNN layer (6-hop sparse message passing + softmax mix + dense proj) on
8 TRN2 NeuronCores — v8 (1.13ms, vs 1.96ms baseline).

Structure:
  - Edges sharded by dest row across 8 cores; per core sorted into
    (supertile of 14 x 128-row tiles, col-chunk) GROUPS and packed into
    128-edge blocks with NO per-tile padding: blocks may straddle tile
    boundaries; a straddling block gets one matmul per tile it touches
    (pair plan), each with its own masked one-hot P. Only group-level
    padding remains (B = max over cores, shared SPMD program): ~458k
    descriptors vs 497k with per-tile padding.
  - SWDGE dma_gather in 1024-idx calls (HW cap; full calls make full
    16KB per-engine DMA packets). The kernel is bound by the SERIAL
    ~2.4us/call SWDGE descriptor-generation cost, so descriptor/call
    count is the primary lever. c3-group calls are capped at 4 blocks
    to bound the worst-case pair buffer.
  - P one-hot scatter matrices built by a runtime-registered fused
    custom-DVE op: eq(slot, Idx - PageIdx(0, W)) * sval over
    [128, pairs, W] in ONE instruction per gather call.
  - Per-(tile, chunk) contiguous PSUM matmul chains (PE accumulation
    chains MUST NOT interleave - interleaved chains in one bank
    corrupt), flushed into an SBUF f32 supertile accumulator (DVE add),
    consumed by per-tile epilogues (proj + bias + PE transpose for
    row-major rep) that stream output DMAs throughout the kernel.
"""

import numpy as np
import ml_dtypes

N = 100000
NPAD = 100352          # 784 * 128
D = 128
NCORES = 8
RPC = NPAD // NCORES   # 12544 rows per core
TILEW = 128
NT = RPC // TILEW      # 98 row tiles per core
S = 14                 # tiles per supertile
NST = NT // S          # 7
NCH = 4
CH_BASE = (0, 32768, 65536, 98304)
CH_SIZE = (32768, 32768, 32768, 2048)
NQ = 4                 # SWDGE queues
CALLB = 8              # blocks per gather call (1024 idx = HW max)
GBUFS = 18             # gather call buffers in flight
PBUFS = 8              # P call buffers in flight

_cache = {}

bf16 = ml_dtypes.bfloat16

_ONEHOT = None


def _register_onehot():
    """Register the fused one-hot-times-scale DVE op:
    out[p, s, j] = (in0[p, s, j] == (k - (s0 + s*s1))) * in1[p, s, j]
    with k the global element index. With s0=0, s1=N (page width), the
    comparison target is the within-page index j."""
    global _ONEHOT
    if _ONEHOT is not None:
        return _ONEHOT
    import concourse.dve_ops as dve_ops
    from concourse.dve_ops import DveOp, OPS, _CUSTOM_DVE_ROW_BASE
    from concourse.dve_spec import Spec, Src0, Src1, C0, C1, eq, Idx, PageIdx, lower
    from concourse.dve_uop import DveOpSpec

    name = "ONEHOT_SVAL_ANT"
    if name in dve_ops._SUB_OPCODE_FOR_NAME:
        _ONEHOT = next(op for op in OPS if op.name == name)
        return _ONEHOT

    def _onehot_ref(in0, in1, s0, s1, imm2):
        P = in0.shape[0]
        Sd = int(np.prod(in0.shape[1:-1]))
        Nd = in0.shape[-1]
        slot = in0.reshape(P, Sd, Nd).astype(np.float32)
        sval = in1.reshape(P, Sd, Nd).astype(np.float32)
        k = np.arange(Sd * Nd, dtype=np.float32).reshape(1, Sd, Nd)
        s0v = s0[:, None] if isinstance(s0, np.ndarray) else s0
        s1v = float(s1.flat[0]) if isinstance(s1, np.ndarray) else s1
        pg = s0v + np.arange(Sd, dtype=np.float32)[None, :, None] * s1v
        return ((slot == (k - pg)) * sval).reshape(in1.shape)

    spec = Spec(body=eq(Src0, Idx - PageIdx(C0, C1)) * Src1, reference=_onehot_ref)
    row = _CUSTOM_DVE_ROW_BASE + len(OPS)
    shas = {}
    for ver in ("v3", "v4"):
        s = DveOpSpec(name=name, opcode=row, uops=lower(spec, ver=ver), rd1_en=True)
        shas[ver] = s.sha(ver)
    op = DveOp(name, spec, subdim=True, uops_sha=shas)
    OPS.append(op)
    dve_ops.CUSTOM_DVE_SPECS[name] = spec
    dve_ops._SUB_OPCODE_FOR_NAME[name] = row
    _ONEHOT = op
    return op


def _prep(input, adj_rows, adj_cols, adj_vals, weight, linear_weight, bias):
    f32 = np.float32

    lw = np.asarray(linear_weight, np.float64)
    e = np.exp(lw - lw.max())
    mix = (e / e.sum()).astype(f32)
    mix0 = float(mix[0])

    rows = np.asarray(adj_rows).reshape(-1)
    cols = np.asarray(adj_cols).reshape(-1)
    sval = (np.asarray(adj_vals, f32) * mix[1:, None]).reshape(-1)

    core = rows // RPC
    NGRP = NST * NCH

    per_core = []
    gcounts = np.zeros((NCORES, NGRP), np.int64)
    for k in range(NCORES):
        m = core == k
        r = (rows[m] - k * RPC).astype(np.int32)
        c = cols[m].astype(np.int32)
        v = sval[m]
        t = r >> 7
        ch = np.minimum(c >> 15, 3)
        st = t // S
        grp = st * NCH + ch
        order = np.lexsort((t, grp))   # by group, then tile within group
        gs = grp[order]
        gcounts[k] = np.bincount(gs, minlength=NGRP)
        per_core.append((r[order], c[order], ch[order], v[order], gs, t[order]))

    Bg = np.maximum(np.ceil(gcounts.max(axis=0) / 128).astype(np.int64), 1)
    gblk0 = np.concatenate([[0], np.cumsum(Bg)])   # group -> first global block
    nblk = int(Bg.sum())
    epad = nblk * 128

    # pair plan: per global block, the union tile range over cores
    tlo = np.full(nblk, 10 ** 9, np.int64)
    thi = np.full(nblk, -1, np.int64)
    for k in range(NCORES):
        r, c, ch, v, gs, ts = per_core[k]
        ncnt = gcounts[k]
        pos = np.arange(len(r)) - np.repeat(
            np.concatenate([[0], np.cumsum(ncnt)[:-1]]), ncnt)
        gblk = gblk0[gs] + (pos >> 7)
        np.minimum.at(tlo, gblk, ts)
        np.maximum.at(thi, gblk, ts)
    # guard: blocks never touched (shouldn't happen) -> tile of their group
    for g in range(NGRP):
        st = g // NCH
        for b in range(int(gblk0[g]), int(gblk0[g + 1])):
            if thi[b] < 0:
                tlo[b] = thi[b] = st * S
    npair_blk = thi - tlo + 1
    pair0 = np.concatenate([[0], np.cumsum(npair_blk)])  # block -> first pair
    npairs = int(pair0[-1])

    xin = np.zeros((NPAD, D), bf16)
    xin[:N] = np.asarray(input, f32).astype(bf16)
    xlocT_f = np.zeros((D, NPAD), f32)
    xlocT_f[:, :N] = np.asarray(input, f32).T

    wmat = np.asarray(weight, f32).astype(bf16)
    bias_b = np.asarray(bias, f32).astype(bf16)[None, :]
    ones_b = np.ones((1, D), bf16)
    ident = np.eye(D, dtype=bf16)

    in_maps = []
    for k in range(NCORES):
        r, c, ch, v, gs, ts = per_core[k]
        ncnt = gcounts[k]
        pos = np.arange(len(r)) - np.repeat(
            np.concatenate([[0], np.cumsum(ncnt)[:-1]]), ncnt)
        gblk = gblk0[gs] + (pos >> 7)
        lane = pos & 127

        cpad = np.zeros(epad, np.int32)
        cbase = np.asarray(CH_BASE, np.int32)[ch]
        cpad[gblk * 128 + lane] = c - cbase

        pidx = pair0[gblk] + (ts - tlo[gblk])
        pslot = np.full(npairs * 128, -1.0, f32)
        psval = np.zeros(npairs * 128, f32)
        pslot[pidx * 128 + lane] = (r & (TILEW - 1)).astype(f32)
        psval[pidx * 128 + lane] = v

        gidx16 = cpad.reshape(-1, 16).T.astype(np.int16)
        gidx = np.broadcast_to(gidx16, (8, 16, epad // 16)).reshape(128, epad // 16).copy()

        in_maps.append({
            "xin": xin,
            "gidx": gidx,
            "slot": pslot.reshape(npairs, 128).T.astype(bf16).copy(),  # [128, npairs]
            "sval": psval.reshape(npairs, 128).T.astype(bf16).copy(),
            "xlocT": np.ascontiguousarray(
                xlocT_f[:, k * RPC:(k + 1) * RPC].astype(bf16)
            ),
            "wmat": wmat,
            "biasb": bias_b,
            "onesb": ones_b,
            "ident": ident,
        })
    plan = (Bg, tlo, thi)
    return in_maps, plan, mix0


def _build(plan, mix0):
    import concourse.bass as bass
    import concourse.bacc as bacc
    import concourse.mybir as mybir
    import concourse.tile as tile

    onehot_op = _register_onehot()
    Bg, tlo, thi = plan

    dt = mybir.dt
    alu = mybir.AluOpType
    nblk = int(Bg.sum())
    epad = nblk * 128
    gblk0 = np.concatenate([[0], np.cumsum(Bg)])
    pair0 = np.concatenate([[0], np.cumsum(thi - tlo + 1)])
    npairs = int(pair0[-1])

    nc = bacc.Bacc(None, num_swdge_queues=NQ)
    xin_d = nc.declare_dram_parameter("xin", [NPAD, D], dt.bfloat16, isOutput=False)
    gidx_d = nc.declare_dram_parameter("gidx", [128, epad // 16], dt.int16, isOutput=False)
    slot_d = nc.declare_dram_parameter("slot", [128, npairs], dt.bfloat16, isOutput=False)
    sval_d = nc.declare_dram_parameter("sval", [128, npairs], dt.bfloat16, isOutput=False)
    xlocT_d = nc.declare_dram_parameter("xlocT", [128, RPC], dt.bfloat16, isOutput=False)
    wmat_d = nc.declare_dram_parameter("wmat", [D, D], dt.bfloat16, isOutput=False)
    bias_d = nc.declare_dram_parameter("biasb", [1, D], dt.bfloat16, isOutput=False)
    ones_d = nc.declare_dram_parameter("onesb", [1, D], dt.bfloat16, isOutput=False)
    ident_d = nc.declare_dram_parameter("ident", [D, D], dt.bfloat16, isOutput=False)
    out_d = nc.declare_dram_parameter("out", [RPC, D], dt.float32, isOutput=True)
    rep_d = nc.declare_dram_parameter("rep", [RPC, D], dt.float32, isOutput=True)

    # calls: per (st, c) group, slices of <=CALLB blocks
    calls = []  # (g, block0_global, cb)
    for g in range(NST * NCH):
        b0, b1 = int(gblk0[g]), int(gblk0[g + 1])
        # c3 blocks straddle many tiles (many pairs): smaller calls keep
        # the worst-case P buffer bounded
        cbmax = 5 if g % NCH == NCH - 1 else CALLB
        s0 = b0
        while s0 < b1:
            cb = min(cbmax, b1 - s0)
            calls.append((g, s0, cb))
            s0 += cb
    qload = [0] * NQ
    qassign = []
    for _, _, cb in calls:
        q = min(range(NQ), key=lambda i: qload[i])
        qassign.append(q)
        qload[q] += cb

    with tile.TileContext(nc) as tc:
        with (
            tc.tile_pool(name="const", bufs=1) as const,
            tc.tile_pool(name="adj", bufs=1) as adj,
            tc.tile_pool(name="gbuf", bufs=GBUFS) as gbuf,
            tc.tile_pool(name="pbuf", bufs=PBUFS) as pbuf,
            tc.tile_pool(name="racc", bufs=2) as racc,
            tc.tile_pool(name="rbuf", bufs=6) as rbuf,
            tc.tile_pool(name="ps_acc", bufs=4, space="PSUM") as ps_acc,
            tc.tile_pool(name="ps_out", bufs=2, space="PSUM") as ps_out,
            tc.tile_pool(name="ps_rep", bufs=2, space="PSUM") as ps_rep,
        ):
            wmat = const.tile([D, D], dt.bfloat16)
            biasb = const.tile([1, D], dt.bfloat16)
            onesb = const.tile([1, D], dt.bfloat16)
            ident = const.tile([D, D], dt.bfloat16)
            xlocT = const.tile([128, RPC], dt.bfloat16)
            gidx = adj.tile([128, epad // 16], dt.int16)
            slot = adj.tile([128, npairs], dt.bfloat16)
            sval = adj.tile([128, npairs], dt.bfloat16)

            nc.sync.dma_start(wmat[:], wmat_d[:])
            nc.sync.dma_start(biasb[:], bias_d[:])
            nc.sync.dma_start(onesb[:], ones_d[:])
            nc.sync.dma_start(ident[:], ident_d[:])
            nc.sync.dma_start(xlocT[:], xlocT_d[:])
            # split index loads per supertile so the first gather calls
            # depend only on the first slice, not the whole load
            for st in range(NST):
                b0 = int(gblk0[st * NCH])
                b1 = int(gblk0[min((st + 1) * NCH, NST * NCH)])
                p0, p1 = int(pair0[b0]), int(pair0[b1])
                nc.sync.dma_start(gidx[:, b0 * 8:b1 * 8], gidx_d[:, b0 * 8:b1 * 8])
                nc.sync.dma_start(slot[:, p0:p1], slot_d[:, p0:p1])
                nc.sync.dma_start(sval[:, p0:p1], sval_d[:, p0:p1])

            # all SWDGE gather calls upfront (ring-buffered)
            call_gt = []
            for ci, (g, b0, cb) in enumerate(calls):
                c = g % NCH
                gt = gbuf.tile([128, cb * 128], dt.bfloat16, tag="gt")
                gt3 = gt[:].rearrange("p (b e) -> p b e", e=128)
                nc.gpsimd.dma_gather(
                    out_ap=gt3,
                    in_ap=xin_d[CH_BASE[c]:CH_BASE[c] + CH_SIZE[c], :],
                    idxs_ap=gidx[:, b0 * 8:(b0 + cb) * 8],
                    num_idxs=cb * 128,
                    num_idxs_reg=cb * 128,
                    elem_size=D,
                    queue_num=qassign[ci],
                )
                call_gt.append(gt3)

            # per supertile: per-(tile, chunk) contiguous PSUM chains over
            # the pairs (block, tile) covering that tile, flushed into an
            # SBUF f32 supertile accumulator; per-call P-builds (one fused
            # DVE op over the call's PAIRS) emitted at first use
            blk2call = {}
            for ci, (g, b0, cb) in enumerate(calls):
                for b in range(b0, b0 + cb):
                    blk2call[b] = ci
            ptbuilt = {}

            def get_pt(ci):
                if ci not in ptbuilt:
                    cg, cb0, ccb = calls[ci]
                    pp0 = int(pair0[cb0])
                    pp1 = int(pair0[cb0 + ccb])
                    npc = pp1 - pp0
                    pt = pbuf.tile([128, npc * TILEW], dt.bfloat16, tag="pt")
                    pt3 = pt[:].rearrange("p (b j) -> p b j", j=TILEW)
                    sl = slot[:, pp0:pp1]
                    sv = sval[:, pp0:pp1]
                    sl_ap = bass.AP(
                        sl.tensor, sl.offset, [sl.ap[0], [1, npc], [0, TILEW]])
                    sv_ap = bass.AP(
                        sv.tensor, sv.offset, [sv.ap[0], [1, npc], [0, TILEW]])
                    nc.vector._custom_dve(
                        onehot_op, out=pt3, in0=sl_ap, in1=sv_ap,
                        s0=0.0, s1=float(TILEW),
                    )
                    ptbuilt[ci] = (pt3, pp0)
                return ptbuilt[ci]

            for st in range(NST):
                rT = racc.tile([128, S * TILEW], dt.float32, tag="rT")
                for c in range(NCH):
                    g = st * NCH + c
                    b0g, b1g = int(gblk0[g]), int(gblk0[g + 1])
                    for q in range(S):
                        t = st * S + q
                        blks = [b for b in range(b0g, b1g) if tlo[b] <= t <= thi[b]]
                        if not blks:
                            continue
                        acc = ps_acc.tile([128, TILEW], dt.float32)
                        for j, b in enumerate(blks):
                            ci = blk2call[b]
                            cg, cb0, ccb = calls[ci]
                            pt3, pp0 = get_pt(ci)
                            p = int(pair0[b]) + (t - int(tlo[b]))
                            nc.tensor.matmul(
                                acc[:], call_gt[ci][:, b - cb0, :],
                                pt3[:, p - pp0, :],
                                start=(j == 0), stop=(j == len(blks) - 1),
                            )
                        rslc = rT[:, q * TILEW:(q + 1) * TILEW]
                        if c == 0:
                            nc.vector.tensor_copy(rslc, acc[:])
                        else:
                            nc.vector.tensor_add(rslc, rslc, acc[:])

                for q in range(S):
                    t = st * S + q
                    rbf = rbuf.tile([D, D], dt.bfloat16, tag="rbf")
                    nc.vector.scalar_tensor_tensor(
                        rbf[:], xlocT[:, t * TILEW:(t + 1) * TILEW], mix0,
                        rT[:, q * TILEW:(q + 1) * TILEW], alu.mult, alu.add,
                    )
                    outp = ps_out.tile([D, D], dt.float32)
                    nc.tensor.matmul(outp[:], rbf[:], wmat[:], start=True, stop=False)
                    nc.tensor.matmul(outp[:], onesb[:], biasb[:], start=False, stop=True)
                    repp = ps_rep.tile([D, D], dt.float32)
                    nc.tensor.matmul(repp[:], rbf[:], ident[:], start=True, stop=True)
                    outs = rbuf.tile([D, D], dt.float32, tag="outs")
                    reps = rbuf.tile([D, D], dt.float32, tag="reps")
                    nc.scalar.copy(outs[:], outp[:])
                    nc.scalar.copy(reps[:], repp[:])
                    r0 = t * TILEW
                    nc.sync.dma_start(out_d[r0:r0 + TILEW, :], outs[:])
                    nc.sync.dma_start(rep_d[r0:r0 + TILEW, :], reps[:])

    nc.compile()
    return nc


def kernel(**inputs):
    from concourse.bass_utils import run_bass_kernel_spmd

    in_maps, plan, mix0 = _prep(**inputs)
    key = (tuple(plan[0]), tuple(plan[1]), tuple(plan[2]), round(mix0, 9))
    if key not in _cache:
        _cache.clear()
        _cache[key] = _build(plan, mix0)
    nc = _cache[key]

    res = run_bass_kernel_spmd(nc, in_maps, list(range(NCORES)))
    out = np.concatenate([np.asarray(res.results[k]["out"]) for k in range(NCORES)])
    rep = np.concatenate([np.asarray(res.results[k]["rep"]) for k in range(NCORES)])
    return out[:N].astype(np.float32), rep[:N].astype(np.float32)
